# revision 5
# baseline (speedup 1.0000x reference)
"""Causal self-attention (B=4, T=2048, C=768, 12 heads) on 8 Trainium2 cores.

Sharding: core i handles batch b = i//2 and head-set s = i%2 (6 of 12 heads).
Each core computes x[b] @ W_attn slice -> 6 heads of causal attention -> a
partial projection (row-sharded W_proj).  The host sums the two partials per
batch and adds b_proj.

Device layout per core (all matmul operands float32r = fp32 storage, FP22
multiply at full PE rate):
  - x^T [768, 2048] (host pre-transposed)
  - Q^T/K^T computed in [head_cols, T] layout via out^T = W_slice.T @ x^T,
    stored as 3 "pair" tiles [128, 2048] (head a on partitions 0-63, head b
    on 64-127); 1/sqrt(64) folded into W_q/b_q on host.
  - V' [2048, 6*65] in natural layout with an all-ones column per head
    (zero weight column + bias 1.0): PV matmul then yields both Y'^T and the
    softmax denominator row.
  - S^T = K^T.T @ Q^T per (head, q-chunk 512, k-chunk 128), causal tiles
    only; exp on ScalarE straight out of PSUM in [128, 1024] groups; the 4
    diagonal tiles per (head, q-chunk) get a 0/1 mask multiply on VectorE.
  - Y'^T [65, 512] accumulated in PSUM over k-chunks; normalized with
    VectorE reciprocal + a K=1 ones matmul broadcast.
  - Partial out = Y_norm @ W_proj_rows accumulated over the 3 pairs.
"""

import numpy as np

import concourse.bass as bass
import concourse.mybir as mybir
import concourse.tile as tile
from concourse import bacc

B, T, C = 4, 2048, 768
NH, HD = 12, 64
N_CORES = 8
HPC = 6  # heads per core
P = 128
F32 = mybir.dt.float32
F32R = mybir.dt.float32r
QC_N = T // 512  # 4 q-chunks of 512
KC_N = T // P    # 16 k-chunks of 128
CKC = C // P     # 6 contraction chunks for the QKV projection


def build_program(n_iters: int = 1):
    """Builds the SPMD program (identical on all cores; data differs)."""
    nc = bacc.Bacc(
        "TRN2",
        target_bir_lowering=False,
        debug=False,
        enable_asserts=False,
        num_devices=N_CORES,
    )
    d_xt = nc.dram_tensor("xt", [C, T], F32R, kind="ExternalInput").ap()
    d_wq = nc.dram_tensor("wq", [C, 384], F32R, kind="ExternalInput").ap()
    d_wk = nc.dram_tensor("wk", [C, 384], F32R, kind="ExternalInput").ap()
    d_wv = nc.dram_tensor("wv", [C, 390], F32R, kind="ExternalInput").ap()
    d_w2 = nc.dram_tensor("w2", [384, C], F32R, kind="ExternalInput").ap()
    d_bq = nc.dram_tensor("bq", [P, 3], F32, kind="ExternalInput").ap()
    d_bk = nc.dram_tensor("bk", [P, 3], F32, kind="ExternalInput").ap()
    d_bv = nc.dram_tensor("bv", [1, 390], F32R, kind="ExternalInput").ap()
    d_ones = nc.dram_tensor("ones", [1, P], F32R, kind="ExternalInput").ap()
    d_masks = nc.dram_tensor("masks", [P, 4 * 512], F32R, kind="ExternalInput").ap()
    d_out = nc.dram_tensor("out", [T, C], F32R, kind="ExternalOutput").ap()

    with tile.TileContext(nc) as tc:
        # Persistent pools (live for the whole body).  PSUM budget (8 banks):
        #   tag "ps_A" [128,1024] x2 = 4 banks  (S^T staging, then proj out)
        #   tag "ps_B" [128,512]  x3 = 3 banks  (QKV transients, Y' accum)
        #   tag "ps_b" [128,512]  x1 = 1 bank   (broadcast scratch)
        const_cm = tc.tile_pool(name="const", bufs=1)
        work_cm = tc.tile_pool(name="work", bufs=1)
        sb_cm = tc.tile_pool(name="sbw", bufs=2)
        ps_cm = tc.tile_pool(name="psum", bufs=1, space="PSUM")
        const = const_cm.__enter__()
        work = work_cm.__enter__()
        sbw = sb_cm.__enter__()
        psp = ps_cm.__enter__()

        def body(_i=None):
            # ---- constant loads (re-emitted per loop iter; cheap) ----
            wq_sb = [const.tile([P, 384], F32R, tag=f"wq{k}", name=f"wq{k}") for k in range(CKC)]
            wk_sb = [const.tile([P, 384], F32R, tag=f"wk{k}", name=f"wk{k}") for k in range(CKC)]
            wv_sb = [const.tile([P, 390], F32R, tag=f"wv{k}", name=f"wv{k}") for k in range(CKC)]
            w2_sb = [const.tile([P, C], F32R, tag=f"w2{p}", name=f"w2{p}") for p in range(3)]
            bq_sb = const.tile([P, 3], F32, tag="bq")
            bk_sb = const.tile([P, 3], F32, tag="bk")
            bv_sb = const.tile([1, 390], F32R, tag="bv")
            ones_sb = const.tile([1, P], F32R, tag="ones")
            masks_sb = const.tile([P, 4 * 512], F32R, tag="masks")
            for k in range(CKC):
                nc.sync.dma_start(wq_sb[k][:], d_wq[k * P:(k + 1) * P, :])
                nc.sync.dma_start(wk_sb[k][:], d_wk[k * P:(k + 1) * P, :])
                nc.sync.dma_start(wv_sb[k][:], d_wv[k * P:(k + 1) * P, :])
            for p in range(3):
                nc.sync.dma_start(w2_sb[p][:], d_w2[p * P:(p + 1) * P, :])
            nc.sync.dma_start(bq_sb[:], d_bq[:])
            nc.sync.dma_start(bk_sb[:], d_bk[:])
            nc.sync.dma_start(bv_sb[:], d_bv[:])
            nc.sync.dma_start(ones_sb[:], d_ones[:])
            nc.sync.dma_start(masks_sb[:], d_masks[:])

            xt_sb = [work.tile([P, T], F32R, tag=f"xt{k}", name=f"xt{k}") for k in range(CKC)]
            for k in range(CKC):
                nc.sync.dma_start(xt_sb[k][:], d_xt[k * P:(k + 1) * P, :])

            qt_sb = [work.tile([P, T], F32R, tag=f"qt{p}", name=f"qtp{p}") for p in range(3)]
            kt_sb = [work.tile([P, T], F32R, tag=f"kt{p}", name=f"ktp{p}") for p in range(3)]
            v_sb = [work.tile([P, 390], F32R, tag=f"v{t}", name=f"v{t}") for t in range(KC_N)]
            # yn reuses the xt slots (xt is dead once QKV is done)
            yn_sb = [work.tile([P, T], F32R, tag=f"xt{p}", name=f"yn{p}")
                     for p in range(3)]

            # ---- QKV projections ----
            # Q^T / K^T per pair: out^T[pair_cols, T]
            for p in range(3):
                for (w_sb, b_sb, o_sb) in ((wq_sb, bq_sb, qt_sb), (wk_sb, bk_sb, kt_sb)):
                    for qc in range(QC_N):
                        ps = psp.tile([P, 512], F32, tag="ps_B", bufs=3)
                        for k in range(CKC):
                            nc.tensor.matmul(
                                ps[:],
                                lhsT=w_sb[k][:, p * P:(p + 1) * P],
                                rhs=xt_sb[k][:, qc * 512:(qc + 1) * 512],
                                start=(k == 0),
                                stop=(k == CKC - 1),
                            )
                        nc.vector.tensor_scalar(
                            o_sb[p][:, qc * 512:(qc + 1) * 512],
                            ps[:],
                            b_sb[:, p:p + 1],
                            None,
                            mybir.AluOpType.add,
                        )
            # V' natural layout [T, 390] (+ones cols via bias row)
            for t in range(KC_N):
                ps = psp.tile([P, 512], F32, tag="ps_B", bufs=3)
                for k in range(CKC):
                    nc.tensor.matmul(
                        ps[:, :390],
                        lhsT=xt_sb[k][:, t * P:(t + 1) * P],
                        rhs=wv_sb[k][:],
                        start=(k == 0),
                        stop=False,
                    )
                nc.tensor.matmul(
                    ps[:, :390], lhsT=ones_sb[:, :P], rhs=bv_sb[:],
                    start=False, stop=True,
                )
                nc.vector.tensor_copy(v_sb[t][:], ps[:, :390])

            # ---- causal attention, 6 heads ----
            for qc in range(QC_N):
                n_kc = 4 * qc + 4
                for p in range(3):
                    for h2 in range(2):
                        ch = p * 2 + h2      # head index within core
                        pb = 64 * h2         # partition base in pair tiles
                        yp = psp.tile([P, 512], F32, tag="ps_B", bufs=3)
                        for g in range(n_kc // 2):
                            ss = psp.tile([P, 1024], F32, tag="ps_A", bufs=2)
                            for j in range(2):
                                kc = 2 * g + j
                                nc.tensor.matmul(
                                    ss[:, j * 512:(j + 1) * 512],
                                    lhsT=kt_sb[p][pb:pb + 64, kc * P:(kc + 1) * P],
                                    rhs=qt_sb[p][pb:pb + 64, qc * 512:(qc + 1) * 512],
                                    start=True,
                                    stop=True,
                                )
                            pt = sbw.tile([P, 1024], F32R, tag="pt", bufs=2)
                            nc.scalar.activation(
                                pt[:], ss[:], mybir.ActivationFunctionType.Exp
                            )
                            for j in range(2):
                                kc = 2 * g + j
                                m = kc - 4 * qc
                                if m >= 0:  # diagonal tile: apply causal mask
                                    nc.vector.tensor_tensor(
                                        pt[:, j * 512:(j + 1) * 512],
                                        pt[:, j * 512:(j + 1) * 512],
                                        masks_sb[:, m * 512:(m + 1) * 512],
                                        mybir.AluOpType.mult,
                                    )
                            for j in range(2):
                                kc = 2 * g + j
                                nc.tensor.matmul(
                                    yp[:65, :],
                                    lhsT=v_sb[kc][:, ch * 65:(ch + 1) * 65],
                                    rhs=pt[:, j * 512:(j + 1) * 512],
                                    start=(kc == 0),
                                    stop=(kc == n_kc - 1),
                                )
                        # normalize: divide rows 0..63 by the denominator row 64
                        recip = sbw.tile([1, 512], F32R, tag="recip", bufs=2)
                        with nc.allow_low_precision("f32r is fp32 storage"):
                            nc.vector.reciprocal(recip[:], yp[64:65, :])
                        bc_ps = psp.tile([P, 512], F32, tag="ps_b", bufs=1)
                        nc.tensor.matmul(
                            bc_ps[:64, :], lhsT=ones_sb[:, :64], rhs=recip[:],
                            start=True, stop=True,
                        )
                        bcs = sbw.tile([64, 512], F32R, tag="bcs", bufs=2)
                        nc.vector.tensor_copy(bcs[:], bc_ps[:64, :])
                        nc.vector.tensor_tensor(
                            yn_sb[p][pb:pb + 64, qc * 512:(qc + 1) * 512],
                            yp[:64, :],
                            bcs[:],
                            mybir.AluOpType.mult,
                        )

            # ---- output projection (partial; host adds pairs + b_proj) ----
            for qb in range(KC_N):
                po = psp.tile([P, 1024], F32, tag="ps_A", bufs=2)
                for (n0, nw) in ((0, 512), (512, 256)):
                    for p in range(3):
                        nc.tensor.matmul(
                            po[:, n0:n0 + nw],
                            lhsT=yn_sb[p][:, qb * P:(qb + 1) * P],
                            rhs=w2_sb[p][:, n0:n0 + nw],
                            start=(p == 0),
                            stop=(p == 2),
                        )
                ob = sbw.tile([P, C], F32R, tag="ob", bufs=2)
                nc.vector.tensor_copy(ob[:], po[:, :C])
                nc.sync.dma_start(d_out[qb * P:(qb + 1) * P, :], ob[:])

        if n_iters == 1:
            body()
        else:
            with tc.For_i(0, n_iters, 1) as _i:
                body(_i)

        for cm in (ps_cm, sb_cm, work_cm, const_cm):
            cm.__exit__(None, None, None)

    nc.compile()
    return nc


def shard_inputs(x, W_attn, b_attn, W_proj, b_proj):
    """Builds the 8 per-core input maps (all host-side numpy prep)."""
    x = np.asarray(x, dtype=np.float32)
    W_attn = np.asarray(W_attn, dtype=np.float32)
    b_attn = np.asarray(b_attn, dtype=np.float32)
    W_proj = np.asarray(W_proj, dtype=np.float32)
    scale = float(HD) ** -0.5

    kl = np.arange(P)[:, None]
    ql = np.arange(512)[None, :]
    masks = np.concatenate(
        [(kl <= ql - 128 * m).astype(np.float32) for m in range(4)], axis=1
    )  # [128, 4*512]
    ones_row = np.ones((1, P), dtype=np.float32)

    in_maps = []
    for core in range(N_CORES):
        b = core // 2
        s = core % 2
        heads = [s * HPC + j for j in range(HPC)]
        xt = np.ascontiguousarray(x[b].T)  # [C, T]

        wq = np.empty((C, 384), np.float32)
        wk = np.empty((C, 384), np.float32)
        bq = np.empty((P, 3), np.float32)
        bk = np.empty((P, 3), np.float32)
        for p in range(3):
            for h2 in range(2):
                hh = heads[p * 2 + h2]
                cols = slice(hh * HD, (hh + 1) * HD)
                dst = slice(h2 * HD, (h2 + 1) * HD)
                wq[:, p * P + h2 * HD:p * P + (h2 + 1) * HD] = (
                    W_attn[:, cols] * scale
                )
                wk[:, p * P + h2 * HD:p * P + (h2 + 1) * HD] = (
                    W_attn[:, C + hh * HD:C + (hh + 1) * HD]
                )
                bq[dst, p] = b_attn[hh * HD:(hh + 1) * HD] * scale
                bk[dst, p] = b_attn[C + hh * HD:C + (hh + 1) * HD]

        wv = np.zeros((C, 390), np.float32)
        bv = np.zeros((1, 390), np.float32)
        for ch in range(HPC):
            hh = heads[ch]
            wv[:, ch * 65:ch * 65 + HD] = W_attn[:, 2 * C + hh * HD:2 * C + (hh + 1) * HD]
            bv[0, ch * 65:ch * 65 + HD] = b_attn[2 * C + hh * HD:2 * C + (hh + 1) * HD]
            bv[0, ch * 65 + HD] = 1.0

        w2 = np.empty((384, C), np.float32)
        for p in range(3):
            for h2 in range(2):
                hh = heads[p * 2 + h2]
                w2[p * P + h2 * HD:p * P + (h2 + 1) * HD, :] = (
                    W_proj[hh * HD:(hh + 1) * HD, :]
                )

        in_maps.append({
            "xt": xt, "wq": wq, "wk": wk, "wv": wv, "w2": w2,
            "bq": bq, "bk": bk, "bv": bv, "ones": ones_row, "masks": masks,
        })
    return in_maps


def unshard_outputs(results, b_proj):
    b_proj = np.asarray(b_proj, dtype=np.float32)
    out = np.empty((B, T, C), np.float32)
    for b in range(B):
        out[b] = results[2 * b]["out"] + results[2 * b + 1]["out"] + b_proj
    return out


_CACHED_NC = None


def kernel(x, W_attn, b_attn, W_proj, b_proj):
    global _CACHED_NC
    from concourse import bass_utils

    if _CACHED_NC is None:
        _CACHED_NC = build_program(1)
    in_maps = shard_inputs(x, W_attn, b_attn, W_proj, b_proj)
    res = bass_utils.run_bass_kernel_spmd(
        _CACHED_NC, in_maps, core_ids=list(range(N_CORES))
    )
    return unshard_outputs(res.results, b_proj)
